# revision 1
# baseline (speedup 1.0000x reference)
"""Chamfer distance loss kernel for 8 Trainium2 NeuronCores.

Strategy:
  loss = mean_i min_j ||pred_i - target_j||            (pred, target: 16384 x 3)

  d2[i,j] = |p_i|^2 + |t_j|^2 - 2 p_i . t_j  is computed as ONE K=24 matmul
  per (128-pred-row, 512-target) tile: host-side we build an augmented
  stationary matrix S [24, 16384] from pred and moving matrix M [24, 16384]
  from target such that S^T M = d2 exactly (to ~fp32 precision).  Each fp32
  scalar is split into 3 bf16 limbs (bf16 has fp32's exponent range; 3x8
  mantissa bits ~ fp32's 24), and all limb products with |error| >= 2^-27
  relative are kept:
    - coords (k=0..2): 6 rows each: (q0,c0) (q0,c1) (q1,c0) (q0,c2) (q1,c1)
      (q2,c0), where q = pred[:,k] limbs and c = (-2*target[:,k]) limbs
    - |p|^2 limbs (3 rows) against ones, ones (3 rows) against |t|^2 limbs.
  bf16 products are exact in the PE's fp32 PSUM accumulation.

  Sharding: pred rows across the 8 cores (2048 rows each), target replicated.
  Per core: 16 blocks of 128 pred rows x 8 groups of 2048 targets; each group
  = 4 matmuls into 4 PSUM banks [128, 2048].  The min over targets is the
  bottleneck (PSUM can only be read by DVE/ACT at 1 elem/lane/cycle), so it
  is split across both engines: 3/8 groups are min-reduced by DVE directly
  from PSUM (tensor_scalar op0=min/op1=min with accum_out), 5/8 groups are
  copied fp32->fp16 into SBUF by the Scalar engine, then min-reduced by DVE
  in its 4x perf mode (16-bit, SBUF, single-src).  Per-block partial mins
  [128, 8] are min-combined, relu'd, sqrt'd on device; the host concatenates
  the 8x2048 min distances and takes the mean.
"""

import sys

if "/opt/trn_rl_repo" not in sys.path:
    sys.path.insert(0, "/opt/trn_rl_repo")

from contextlib import ExitStack

import numpy as np
import ml_dtypes

N_CORES = 8
V1 = 16384
V2 = 16384
D = 3
ROWS_PER_CORE = V1 // N_CORES  # 2048
BLOCKS = ROWS_PER_CORE // 128  # 16
GROUP = 2048                   # targets per evacuation group (4 PSUM banks)
GROUPS = V2 // GROUP           # 8
K = 24                         # augmented contraction rows
BIG = 3.0e38                   # identity element for min

_cache: dict = {}


def _build():
    from concourse import bacc, tile, mybir

    f32 = mybir.dt.float32
    bf16 = mybir.dt.bfloat16
    f16 = mybir.dt.float16
    MIN = mybir.AluOpType.min

    nc = bacc.Bacc(
        "TRN2", target_bir_lowering=False, debug=False, num_devices=N_CORES
    )
    sta = nc.dram_tensor("sta", [K, ROWS_PER_CORE], bf16, kind="ExternalInput").ap()
    mov = nc.dram_tensor("mov", [K, V2], bf16, kind="ExternalInput").ap()
    out = nc.dram_tensor("out", [128, BLOCKS], f32, kind="ExternalOutput").ap()

    with tile.TileContext(nc) as tc, ExitStack() as ctx:
        singles = ctx.enter_context(tc.tile_pool(name="singles", bufs=1))
        psump = ctx.enter_context(tc.tile_pool(name="psum", bufs=2, space="PSUM"))
        cpp = ctx.enter_context(tc.tile_pool(name="cpp", bufs=3))
        partp = ctx.enter_context(tc.tile_pool(name="partp", bufs=2))

        sta_sb = singles.tile([K, ROWS_PER_CORE], bf16, tag="sta")
        nc.sync.dma_start(out=sta_sb[:], in_=sta[:])
        # one tile per target group so matmuls only depend on their own DMA
        mov_sb = []
        for g in range(GROUPS):
            mt = singles.tile([K, GROUP], bf16, tag=f"mov{g}")
            nc.sync.dma_start(out=mt[:], in_=mov[:, g * GROUP : (g + 1) * GROUP])
            mov_sb.append(mt)

        allmins = singles.tile([128, BLOCKS], f32, tag="allmins")

        for m in range(BLOCKS):
            bp = partp.tile([128, GROUPS], f32, tag="bp")
            for g in range(GROUPS):
                ps = psump.tile([128, GROUP], f32, tag="ps")
                for j in range(4):
                    nc.tensor.matmul(
                        out=ps[:, 512 * j : 512 * (j + 1)],
                        lhsT=sta_sb[:, 128 * m : 128 * (m + 1)],
                        rhs=mov_sb[g][:, 512 * j : 512 * (j + 1)],
                        start=True,
                        stop=True,
                    )
                if g % 3 == 0:  # 3 of 8 groups: DVE reduces PSUM directly
                    nc.vector.tensor_scalar(
                        out=ps[:],
                        in0=ps[:],
                        scalar1=BIG,
                        scalar2=None,
                        op0=MIN,
                        op1=MIN,
                        accum_out=bp[:, g : g + 1],
                    )
                else:  # 5 of 8: ACT copies PSUM->SBUF fp16, DVE reduces at 4x
                    cp = cpp.tile([128, GROUP], f16, tag="cp")
                    nc.scalar.copy(out=cp[:], in_=ps[:])
                    nc.vector.tensor_scalar(
                        out=cp[:],
                        in0=cp[:],
                        scalar1=BIG,
                        scalar2=None,
                        op0=MIN,
                        op1=MIN,
                        accum_out=bp[:, g : g + 1],
                    )
            nc.vector.tensor_scalar(
                out=bp[:],
                in0=bp[:],
                scalar1=BIG,
                scalar2=None,
                op0=MIN,
                op1=MIN,
                accum_out=allmins[:, m : m + 1],
            )

        nc.vector.tensor_scalar_max(allmins[:], allmins[:], 0.0)
        sq = singles.tile([128, BLOCKS], f32, tag="sq")
        nc.scalar.sqrt(sq[:], allmins[:])
        nc.sync.dma_start(out=out[:], in_=sq[:])

    nc.compile()
    return nc


def _limbs3(x32: np.ndarray):
    """Split fp32 array into 3 bf16 limbs with x ~= l0 + l1 + l2."""
    bf = ml_dtypes.bfloat16
    l0 = x32.astype(bf)
    r1 = x32 - l0.astype(np.float32)
    l1 = r1.astype(bf)
    r2 = r1 - l1.astype(np.float32)
    l2 = r2.astype(bf)
    return l0, l1, l2


def _augment(pred: np.ndarray, target: np.ndarray):
    """Build stationary S [24, V1] (from pred) and moving M [24, V2] (from
    target) bf16 matrices with S^T M ~= pairwise squared distances."""
    bf = ml_dtypes.bfloat16
    S = np.empty((K, V1), dtype=bf)
    M = np.empty((K, V2), dtype=bf)
    for k in range(D):
        q0, q1, q2 = _limbs3(pred[:, k].astype(np.float32))
        c0, c1, c2 = _limbs3((-2.0 * target[:, k]).astype(np.float32))
        r = 6 * k
        S[r + 0], M[r + 0] = q0, c0
        S[r + 1], M[r + 1] = q0, c1
        S[r + 2], M[r + 2] = q1, c0
        S[r + 3], M[r + 3] = q0, c2
        S[r + 4], M[r + 4] = q1, c1
        S[r + 5], M[r + 5] = q2, c0
    p2 = (pred.astype(np.float64) ** 2).sum(axis=1).astype(np.float32)
    t2 = (target.astype(np.float64) ** 2).sum(axis=1).astype(np.float32)
    P0, P1, P2 = _limbs3(p2)
    T0, T1, T2 = _limbs3(t2)
    ones_s = np.ones(V1, dtype=bf)
    ones_m = np.ones(V2, dtype=bf)
    S[18], M[18] = P0, ones_m
    S[19], M[19] = P1, ones_m
    S[20], M[20] = P2, ones_m
    S[21], M[21] = ones_s, T0
    S[22], M[22] = ones_s, T1
    S[23], M[23] = ones_s, T2
    return S, M


def kernel(pred, target) -> np.ndarray:
    from concourse.bass_utils import run_bass_kernel_spmd

    pred = np.asarray(pred, dtype=np.float32)
    target = np.asarray(target, dtype=np.float32)
    assert pred.shape == (V1, D) and target.shape == (V2, D)

    if "nc" not in _cache:
        _cache["nc"] = _build()
    nc = _cache["nc"]

    S, M = _augment(pred, target)
    in_maps = [
        {
            "sta": np.ascontiguousarray(
                S[:, c * ROWS_PER_CORE : (c + 1) * ROWS_PER_CORE]
            ),
            "mov": M,
        }
        for c in range(N_CORES)
    ]
    res = run_bass_kernel_spmd(nc, in_maps, core_ids=list(range(N_CORES)))
    # out[p, m] = min distance of pred row  c*2048 + m*128 + p
    vecs = [res.results[c]["out"].T.reshape(-1) for c in range(N_CORES)]
    mins = np.concatenate(vecs)
    return np.float32(np.mean(mins.astype(np.float64)))


# revision 7
# speedup vs baseline: 1.0019x; 1.0019x over previous
"""Chamfer distance loss kernel for 8 Trainium2 NeuronCores.

  loss = mean_i min_j ||pred_i - target_j||        (pred, target: 16384 x 3)

Algorithm (per core; pred rows sharded 8 ways, target replicated):

1. d2[i,j] via ONE K=24 bf16 matmul per tile: host builds augmented
   stationary S [24, V1] (from pred) and moving M [24, V2] (from target)
   with S^T M = d2 to ~fp32 accuracy (3-limb bf16 splits of the coords,
   -2*coords, |p|^2 and |t|^2; bf16 products are exact in fp32 PSUM).
   The PE runs in 32-row tiling mode (K=24 -> 32): S and M are replicated
   at partition offsets 0/32/64/96 so FOUR matmuls execute concurrently on
   independent PE sub-arrays, one per PSUM bank.

2. The min over targets is evacuation-bound: PSUM is readable only by the
   Vector engine (min-reduce at 1 elem/lane/cycle) and the Scalar engine
   (activation; its accumulator can only SUM).  So the min is split:
   - "exact" groups: DVE tensor_scalar(op0=min, op1=min, accum_out) straight
     from PSUM -> per-block partial mins, combined to m[row] = exact min
     over those groups.
   - "smooth" groups: ACT computes exp(-BETA*(d2 - m)/m) via
     activation(Exp, scale=-BETA/m (per-row AP), bias=+BETA) with
     accum_out summing 2048 terms per instruction.  All exponents are
     <= BETA*(1 + eps-guard) < 87, so no overflow is possible; the row min
     is recovered as m*(1 - relu(log(sum))/BETA), a relative-accuracy
     log-sum-exp min with resolution ~m/BETA (error suppressed by
     exp(-BETA*gap/m); expected loss bias ~ -2e-4 relative).
   Final per-row: min(exact, smooth) -> relu -> sqrt on device; host
   concatenates 8x2048 distances and takes the mean.
"""

import sys

if "/opt/trn_rl_repo" not in sys.path:
    sys.path.insert(0, "/opt/trn_rl_repo")

from contextlib import ExitStack

import numpy as np
import ml_dtypes

N_CORES = 8
V1 = 16384
V2 = 16384
D = 3
ROWS_PER_CORE = V1 // N_CORES  # 2048
BLOCKS = ROWS_PER_CORE // 128  # 16
GROUP = 2048                   # targets per evacuation group (4 PSUM banks)
GROUPS = V2 // GROUP           # 8
K = 24                         # augmented contraction rows
BIG = 3.0e38
BETA = 80.0                    # smooth-min sharpness (exponent cap)
EPSM = 1e-4                    # m floor: caps exponent at BETA*(1+4e-6/EPSM)

_cache: dict = {}


def _build():
    from concourse import bacc, tile, mybir

    f32 = mybir.dt.float32
    bf16 = mybir.dt.bfloat16
    MIN = mybir.AluOpType.min
    ADD = mybir.AluOpType.add
    MULT = mybir.AluOpType.mult
    EXP = mybir.ActivationFunctionType.Exp
    LOG = mybir.ActivationFunctionType.Ln

    nc = bacc.Bacc(
        "TRN2", target_bir_lowering=False, debug=False, num_devices=N_CORES
    )
    sta = nc.dram_tensor("sta", [K, ROWS_PER_CORE], bf16, kind="ExternalInput").ap()
    mov = nc.dram_tensor("mov", [K, V2], bf16, kind="ExternalInput").ap()
    out = nc.dram_tensor("out", [128, BLOCKS], f32, kind="ExternalOutput").ap()

    # exact-share group count per block: ~3.5 avg balances DVE vs ACT
    n_exact = [4 if m % 2 == 0 else 3 for m in range(BLOCKS)]

    with tile.TileContext(nc) as tc, ExitStack() as ctx:
        singles = ctx.enter_context(tc.tile_pool(name="singles", bufs=1))
        psump = ctx.enter_context(tc.tile_pool(name="psum", bufs=2, space="PSUM"))
        partp = ctx.enter_context(tc.tile_pool(name="partp", bufs=3))

        # stationary + moving replicated into the four 32-partition quadrants
        sta_sb = singles.tile([128, ROWS_PER_CORE], bf16, tag="sta")
        for q in range(4):
            eng = nc.sync if q % 2 == 0 else nc.gpsimd
            eng.dma_start(out=sta_sb[32 * q : 32 * q + K, :], in_=sta[:])
        mov_sb = []
        for g in range(GROUPS):
            mt = singles.tile([128, GROUP], bf16, tag=f"mov{g}")
            for q in range(4):
                eng = nc.sync if (g * 4 + q) % 2 == 0 else nc.gpsimd
                eng.dma_start(
                    out=mt[32 * q : 32 * q + K, :],
                    in_=mov[:, g * GROUP : (g + 1) * GROUP],
                )
            mov_sb.append(mt)

        allexact = singles.tile([128, BLOCKS], f32, tag="allexact")
        allmx = singles.tile([128, BLOCKS], f32, tag="allmx")
        alllogs = singles.tile([128, BLOCKS], f32, tag="alllogs")
        bconst = singles.tile([128, 1], f32, tag="bconst")
        nc.vector.memset(bconst[:], BETA)
        zconst = singles.tile([128, 1], f32, tag="zconst")
        nc.vector.memset(zconst[:], 0.0)

        def do_group(ps, m, g):
            for j in range(4):
                q = 32 * j
                nc.tensor.matmul(
                    out=ps[:, 512 * j : 512 * (j + 1)],
                    lhsT=sta_sb[q : q + K, 128 * m : 128 * (m + 1)],
                    rhs=mov_sb[g][q : q + K, 512 * j : 512 * (j + 1)],
                    start=True,
                    stop=True,
                    tile_position=(q, 0),
                )

        for m in range(BLOCKS):
            a = n_exact[m]
            bp = partp.tile([128, GROUPS], f32, tag="bp")
            # exact groups on DVE
            for g in range(a):
                ps = psump.tile([128, GROUP], f32, tag="ps")
                do_group(ps, m, g)
                nc.vector.tensor_scalar(
                    out=ps[:], in0=ps[:], scalar1=BIG, scalar2=None,
                    op0=MIN, op1=MIN, accum_out=bp[:, g : g + 1],
                )
            nc.vector.tensor_scalar(
                out=bp[:, 0:a], in0=bp[:, 0:a], scalar1=BIG, scalar2=None,
                op0=MIN, op1=MIN, accum_out=allexact[:, m : m + 1],
            )
            # scale = -BETA / max(m_exact, EPSM)
            nc.vector.tensor_scalar_max(
                allmx[:, m : m + 1], allexact[:, m : m + 1], EPSM
            )
            rc = partp.tile([128, 1], f32, tag="rc")
            nc.vector.reciprocal(rc[:], allmx[:, m : m + 1])
            sc = partp.tile([128, 1], f32, tag="sc")
            nc.vector.tensor_scalar_mul(sc[:], rc[:], -BETA)
            # smooth groups on ACT
            for g in range(a, GROUPS):
                ps = psump.tile([128, GROUP], f32, tag="ps")
                do_group(ps, m, g)
                nc.scalar.activation(
                    out=ps[:], in_=ps[:], func=EXP,
                    bias=bconst[:], scale=sc[:],
                    accum_out=bp[:, g : g + 1],
                )
            # sum the per-group sums of exp
            nc.vector.tensor_scalar(
                out=bp[:, a:GROUPS], in0=bp[:, a:GROUPS], scalar1=0.0, scalar2=None,
                op0=ADD, op1=ADD, accum_out=alllogs[:, m : m + 1],
            )

        # epilogue: smooth = m * (1 - relu(ln(S))/BETA); out = sqrt(relu(min))
        # The ACT Ln spline is only accurate for inputs up to ~1e15, but S can
        # reach e^84, so compute ln(S) exactly via exponent/mantissa split:
        # ln(S) = (E - 127)*ln2 + ln(mant), mant in [1, 2).
        u32 = mybir.dt.uint32
        LSR = mybir.AluOpType.logical_shift_right
        AND = mybir.AluOpType.bitwise_and
        OR = mybir.AluOpType.bitwise_or
        SUB = mybir.AluOpType.subtract
        ebits = singles.tile([128, BLOCKS], u32, tag="ebits")
        nc.vector.tensor_scalar(
            out=ebits[:], in0=alllogs[:].bitcast(u32), scalar1=23, scalar2=None,
            op0=LSR,
        )
        ef = singles.tile([128, BLOCKS], f32, tag="ef")
        nc.vector.tensor_copy(ef[:], ebits[:])
        mbits = singles.tile([128, BLOCKS], u32, tag="mbits")
        nc.vector.tensor_scalar(
            out=mbits[:], in0=alllogs[:].bitcast(u32),
            scalar1=0x007FFFFF, scalar2=0x3F800000, op0=AND, op1=OR,
        )
        lnm = singles.tile([128, BLOCKS], f32, tag="lnm")
        nc.scalar.activation(
            out=lnm[:], in_=mbits[:].bitcast(f32), func=LOG, bias=zconst[:]
        )
        logs = singles.tile([128, BLOCKS], f32, tag="logs")
        nc.vector.tensor_scalar(
            out=logs[:], in0=ef[:], scalar1=127.0, scalar2=float(np.log(2.0)),
            op0=SUB, op1=MULT,
        )
        nc.vector.tensor_tensor(out=logs[:], in0=logs[:], in1=lnm[:], op=ADD)
        nc.vector.tensor_scalar_max(logs[:], logs[:], 0.0)
        adj = singles.tile([128, BLOCKS], f32, tag="adj")
        nc.vector.tensor_scalar(
            out=adj[:], in0=logs[:], scalar1=-1.0 / BETA, scalar2=1.0,
            op0=MULT, op1=ADD,
        )
        mn = singles.tile([128, BLOCKS], f32, tag="mn")
        # smooth = max(m, EPSM) * adj; result = min(exact, smooth)
        nc.vector.tensor_tensor(out=mn[:], in0=allmx[:], in1=adj[:], op=MULT)
        nc.vector.tensor_tensor(out=mn[:], in0=mn[:], in1=allexact[:], op=MIN)
        nc.vector.tensor_scalar_max(mn[:], mn[:], 0.0)
        sq = singles.tile([128, BLOCKS], f32, tag="sq")
        nc.scalar.sqrt(sq[:], mn[:])
        nc.sync.dma_start(out=out[:], in_=sq[:])

    nc.compile()
    return nc


def _limbs3(x32: np.ndarray):
    """Split fp32 array into 3 bf16 limbs with x ~= l0 + l1 + l2."""
    bf = ml_dtypes.bfloat16
    l0 = x32.astype(bf)
    r1 = x32 - l0.astype(np.float32)
    l1 = r1.astype(bf)
    r2 = r1 - l1.astype(np.float32)
    l2 = r2.astype(bf)
    return l0, l1, l2


def _augment(pred: np.ndarray, target: np.ndarray):
    """Build stationary S [24, V1] (pred) and moving M [24, V2] (target)
    bf16 matrices with S^T M ~= pairwise squared distances."""
    bf = ml_dtypes.bfloat16
    S = np.empty((K, V1), dtype=bf)
    M = np.empty((K, V2), dtype=bf)
    for k in range(D):
        q0, q1, q2 = _limbs3(pred[:, k].astype(np.float32))
        c0, c1, c2 = _limbs3((-2.0 * target[:, k]).astype(np.float32))
        r = 6 * k
        S[r + 0], M[r + 0] = q0, c0
        S[r + 1], M[r + 1] = q0, c1
        S[r + 2], M[r + 2] = q1, c0
        S[r + 3], M[r + 3] = q0, c2
        S[r + 4], M[r + 4] = q1, c1
        S[r + 5], M[r + 5] = q2, c0
    p2 = (pred.astype(np.float64) ** 2).sum(axis=1).astype(np.float32)
    t2 = (target.astype(np.float64) ** 2).sum(axis=1).astype(np.float32)
    P0, P1, P2 = _limbs3(p2)
    T0, T1, T2 = _limbs3(t2)
    ones_s = np.ones(V1, dtype=bf)
    ones_m = np.ones(V2, dtype=bf)
    S[18], M[18] = P0, ones_m
    S[19], M[19] = P1, ones_m
    S[20], M[20] = P2, ones_m
    S[21], M[21] = ones_s, T0
    S[22], M[22] = ones_s, T1
    S[23], M[23] = ones_s, T2
    return S, M


def kernel(pred, target) -> np.ndarray:
    from concourse.bass_utils import run_bass_kernel_spmd

    pred = np.asarray(pred, dtype=np.float32)
    target = np.asarray(target, dtype=np.float32)
    assert pred.shape == (V1, D) and target.shape == (V2, D)

    if "nc" not in _cache:
        _cache["nc"] = _build()
    nc = _cache["nc"]

    S, M = _augment(pred, target)
    in_maps = [
        {
            "sta": np.ascontiguousarray(
                S[:, c * ROWS_PER_CORE : (c + 1) * ROWS_PER_CORE]
            ),
            "mov": M,
        }
        for c in range(N_CORES)
    ]
    res = run_bass_kernel_spmd(nc, in_maps, core_ids=list(range(N_CORES)))
    # out[p, m] = min distance of pred row  c*2048 + m*128 + p
    vecs = [res.results[c]["out"].T.reshape(-1) for c in range(N_CORES)]
    mins = np.concatenate(vecs)
    return np.float32(np.mean(mins.astype(np.float64)))
